# revision 2
# baseline (speedup 1.0000x reference)
"""ColumnParallelLinearWithLoRA kernel for 8 Trainium2 NeuronCores.

Computes out = x @ W.T + bias + 2.0 * lora, where lora routes each token
through one of 8 LoRA adapters (rank 16): lora[s] = B[idx_s] @ (A[idx_s] @ x[s]).

Sharding: data-parallel over tokens (1024 tokens per core). Each core keeps
its x-shard transposed and resident in SBUF, streams the full transposed
weight from HBM exactly once, and computes the LoRA path only for its own
tokens. W/bias/A/B are replicated; the host gathers and transposes the 8
per-core [d_out, tokens] shards back into [S, d_out].

Matmuls run in float32r (fp32 with 11-bit mantissa, 4x the fp32 rate on the
PE): operands are pre-rounded on the host; the LoRA A-projections are rounded
by the DVE when the adapter one-hot mask (built on host) is applied.
"""
from contextlib import ExitStack

import numpy as np

import concourse.bacc as bacc
import concourse.tile as tile
from concourse import mybir
from concourse.bass_utils import run_bass_kernel_spmd

S, D_IN, D_OUT, R, L = 8192, 4096, 4096, 16, 8
SCALING = 2.0
N_CORES = 8
P = 128
S_SH = S // N_CORES          # 1024 tokens per core
LR = L * R                   # 128 stacked adapter rows
KC = D_IN // P               # 32 contraction chunks
TC = S_SH // 512             # 2 token chunks of 512
MB = D_OUT // 512            # 8 output blocks of 512
MC = 4                       # 128-row output chunks per block

f32 = mybir.dt.float32
f32r = mybir.dt.float32r

_CACHE = {}


def round_f32r(a: np.ndarray) -> np.ndarray:
    """Round-to-nearest to fp32r (11 explicit mantissa bits, low 12 bits 0)."""
    u = np.ascontiguousarray(a, dtype=np.float32).view(np.uint32)
    rbit = (u >> np.uint32(12)) & np.uint32(1)
    ur = (u + np.uint32(0x7FF) + rbit) & np.uint32(0xFFFFF000)
    return ur.view(np.float32)


def _build_nc():
    nc = bacc.Bacc("TRN2", target_bir_lowering=False, debug=False,
                   num_devices=N_CORES)
    xt = nc.dram_tensor("xt", [D_IN, S_SH], f32r, kind="ExternalInput").ap()
    wt = nc.dram_tensor("wt", [D_IN, D_OUT], f32r, kind="ExternalInput").ap()
    at = nc.dram_tensor("at", [D_IN, LR], f32r, kind="ExternalInput").ap()
    btf = nc.dram_tensor("btf", [LR, D_OUT], f32r, kind="ExternalInput").ap()
    m1 = nc.dram_tensor("m1", [LR, S_SH], f32, kind="ExternalInput").ap()
    bias = nc.dram_tensor("bias", [D_OUT], f32, kind="ExternalInput").ap()
    ot = nc.dram_tensor("ot", [D_OUT, S_SH], f32, kind="ExternalOutput").ap()

    xt_r = xt.rearrange("(ki p) n -> ki p n", p=P)
    wt_r = wt.rearrange("(ki p) n -> ki p n", p=P)
    at_r = at.rearrange("(ki p) n -> ki p n", p=P)
    ot_r = ot.rearrange("(j p) n -> j p n", p=P)

    with tile.TileContext(nc) as tc, ExitStack() as ctx:
        xpool = ctx.enter_context(tc.tile_pool(name="xres", bufs=KC))
        cpool = ctx.enter_context(tc.tile_pool(name="const", bufs=1))
        apool = ctx.enter_context(tc.tile_pool(name="abar", bufs=TC))
        wpool = ctx.enter_context(tc.tile_pool(name="wstream", bufs=12))
        opool = ctx.enter_context(tc.tile_pool(name="oevict", bufs=4))
        psum = ctx.enter_context(tc.tile_pool(name="ps", bufs=8, space="PSUM"))

        # Residents: x-shard (128 KiB/partition), B-flat, mask, bias.
        xres = []
        for ki in range(KC):
            xk = xpool.tile([P, S_SH], f32r, tag="xres")
            nc.sync.dma_start(xk[:], xt_r[ki])
            xres.append(xk)
        btf_sb = cpool.tile([P, D_OUT], f32r, tag="btf")
        nc.sync.dma_start(btf_sb[:], btf[:])
        m1_sb = cpool.tile([P, S_SH], f32, tag="m1")
        nc.sync.dma_start(m1_sb[:], m1[:])
        bias_sb = cpool.tile([P, D_OUT // P], f32, tag="bias")
        nc.sync.dma_start(bias_sb[:], bias.rearrange("(j p) -> p j", p=P))

        # Phase 1: all-adapter projections a = A_stack @ x, masked per token.
        abar = []
        for t in range(TC):
            pa = psum.tile([P, 512], f32, tag="ps")
            for ki in range(KC):
                atile = wpool.tile([P, P], f32r, tag="astream")
                nc.sync.dma_start(atile[:], at_r[ki])
                nc.tensor.matmul(pa[:], atile[:],
                                 xres[ki][:, t * 512:(t + 1) * 512],
                                 start=(ki == 0), stop=(ki == KC - 1))
            ab = apool.tile([P, 512], f32r, tag="abar")
            nc.vector.tensor_mul(ab[:], pa[:], m1_sb[:, t * 512:(t + 1) * 512])
            abar.append(ab)

        # Phase 2: stream W once; accumulate base GEMM + LoRA B-side in PSUM.
        for mb in range(MB):
            po = [[psum.tile([P, 512], f32, tag="ps", name=f"po{mb}_{mc}_{t}")
                   for t in range(TC)] for mc in range(MC)]
            for ki in range(KC):
                wtile = wpool.tile([P, 512], f32r, tag="wstream")
                nc.sync.dma_start(wtile[:], wt_r[ki][:, mb * 512:(mb + 1) * 512])
                for mc in range(MC):
                    for t in range(TC):
                        nc.tensor.matmul(
                            po[mc][t][:],
                            wtile[:, mc * P:(mc + 1) * P],
                            xres[ki][:, t * 512:(t + 1) * 512],
                            start=(ki == 0), stop=False)
            for mc in range(MC):
                j = mb * MC + mc
                for t in range(TC):
                    nc.tensor.matmul(po[mc][t][:],
                                     btf_sb[:, j * P:(j + 1) * P],
                                     abar[t][:], start=False, stop=True)
                    osb = opool.tile([P, 512], f32, tag="oevict")
                    nc.scalar.activation(osb[:], po[mc][t][:],
                                         mybir.ActivationFunctionType.Identity,
                                         bias=bias_sb[:, j:j + 1])
                    nc.sync.dma_start(
                        ot_r[j][:, t * 512:(t + 1) * 512], osb[:])

    nc.compile()
    return nc


def get_nc():
    if "nc" not in _CACHE:
        _CACHE["nc"] = _build_nc()
    return _CACHE["nc"]


def prep_in_maps(x, weight, bias, A_buffer, B_buffer, weight_indices):
    """Host-side sharding + layout transforms + fp32r pre-rounding."""
    x = np.asarray(x, dtype=np.float32)
    weight = np.asarray(weight, dtype=np.float32)
    bias = np.asarray(bias, dtype=np.float32)
    A_buffer = np.asarray(A_buffer, dtype=np.float32)
    B_buffer = np.asarray(B_buffer, dtype=np.float32)
    weight_indices = np.asarray(weight_indices)

    wt = round_f32r(weight.T)                              # [D_IN, D_OUT]
    at = round_f32r(A_buffer.reshape(LR, D_IN).T)          # [D_IN, LR]
    # btf[l*R+r, m] = B_buffer[0, l, m, r] * SCALING
    btf = round_f32r(B_buffer[0].transpose(0, 2, 1).reshape(LR, D_OUT)
                     * SCALING)
    m1_full = np.repeat(
        (np.arange(L)[:, None] == weight_indices[None, :]), R, axis=0
    ).astype(np.float32)                                   # [LR, S]
    xt_full = round_f32r(x).T                              # view [D_IN, S]

    in_maps = []
    for c in range(N_CORES):
        sl = slice(c * S_SH, (c + 1) * S_SH)
        in_maps.append({
            "xt": np.ascontiguousarray(xt_full[:, sl]),
            "wt": wt,
            "at": at,
            "btf": btf,
            "m1": np.ascontiguousarray(m1_full[:, sl]),
            "bias": bias,
        })
    return in_maps


def gather(results):
    out = np.empty((S, D_OUT), dtype=np.float32)
    for c in range(N_CORES):
        out[c * S_SH:(c + 1) * S_SH, :] = results[c]["ot"].T
    return out


def kernel(x, weight, bias, A_buffer, B_buffer, weight_indices):
    nc = get_nc()
    in_maps = prep_in_maps(x, weight, bias, A_buffer, B_buffer, weight_indices)
    res = run_bass_kernel_spmd(nc, in_maps, list(range(N_CORES)))
    return gather(res.results)


# revision 3
# speedup vs baseline: 1.0976x; 1.0976x over previous
"""ColumnParallelLinearWithLoRA kernel for 8 Trainium2 NeuronCores.

Computes out = x @ W.T + bias + 2.0 * lora, where lora routes each token
through one of 8 LoRA adapters (rank 16): lora[s] = B[idx_s] @ (A[idx_s] @ x[s]).

Sharding: data-parallel over tokens (1024 tokens per core). Each core keeps
its x-shard transposed and resident in SBUF, streams the full transposed
weight from HBM exactly once, and computes the LoRA path only for its own
tokens. W/bias/A/B are replicated; the host gathers and transposes the 8
per-core [d_out, tokens] shards back into [S, d_out].

Matmuls run in float32r (fp32 with 11-bit mantissa, 4x the fp32 rate on the
PE): operands are pre-rounded on the host; the LoRA A-projections are rounded
by the DVE when the adapter one-hot mask (built on host) is applied.

The startup x-load (16.8 MB) is overlapped with compute: the LoRA-A
projections (2 PSUM banks) and 3/4 of the first weight block (6 banks) run
in a ki-loop paced by x arrival; the deferred quarter (mc=3) runs as a small
second pass (mb0c) re-fetching its 2 MB of W. DMA queues are split: x/m1/
bias/btf + even-ki W on the Sync HWDGE queue, packed-A + mb0/mb0c/odd-ki W
on the Scalar HWDGE queue, output evictions on GpSimd SWDGE.
"""
from contextlib import ExitStack

import numpy as np

import concourse.bacc as bacc
import concourse.tile as tile
from concourse import mybir
from concourse.bass_utils import run_bass_kernel_spmd

S, D_IN, D_OUT, R, L = 8192, 4096, 4096, 16, 8
SCALING = 2.0
N_CORES = 8
P = 128
S_SH = S // N_CORES          # 1024 tokens per core
LR = L * R                   # 128 stacked adapter rows
KC = D_IN // P               # 32 contraction chunks
TC = S_SH // 512             # 2 token chunks of 512
MB = D_OUT // 512            # 8 output blocks of 512
MC = 4                       # 128-row output chunks per block

f32 = mybir.dt.float32
f32r = mybir.dt.float32r

_CACHE = {}


def round_f32r(a: np.ndarray) -> np.ndarray:
    """Round-to-nearest to fp32r (11 explicit mantissa bits, low 12 bits 0)."""
    u = np.ascontiguousarray(a, dtype=np.float32).view(np.uint32)
    rbit = (u >> np.uint32(12)) & np.uint32(1)
    ur = (u + np.uint32(0x7FF) + rbit) & np.uint32(0xFFFFF000)
    return ur.view(np.float32)


def _build_nc():
    nc = bacc.Bacc("TRN2", target_bir_lowering=False, debug=False,
                   num_devices=N_CORES)
    xt = nc.dram_tensor("xt", [D_IN, S_SH], f32r, kind="ExternalInput").ap()
    wt = nc.dram_tensor("wt", [D_IN, D_OUT], f32r, kind="ExternalInput").ap()
    at = nc.dram_tensor("at", [P, KC * LR], f32r, kind="ExternalInput").ap()
    btf = nc.dram_tensor("btf", [LR, D_OUT], f32r, kind="ExternalInput").ap()
    m1 = nc.dram_tensor("m1", [LR, S_SH], f32, kind="ExternalInput").ap()
    bias = nc.dram_tensor("bias", [D_OUT], f32, kind="ExternalInput").ap()
    ot = nc.dram_tensor("ot", [D_OUT, S_SH], f32, kind="ExternalOutput").ap()

    # x ki-pair view: [kp, p, j, n] with ki = 2*kp + j
    xt_r = xt.rearrange("(kp j p) n -> kp p j n", p=P, j=2)
    wt_r = wt.rearrange("(ki p) n -> ki p n", p=P)
    ot_r = ot.rearrange("(j p) n -> j p n", p=P)

    with tile.TileContext(nc) as tc, ExitStack() as ctx:
        xpool = ctx.enter_context(tc.tile_pool(name="xres", bufs=KC // 2))
        cpool = ctx.enter_context(tc.tile_pool(name="const", bufs=1))
        apool = ctx.enter_context(tc.tile_pool(name="abar", bufs=TC))
        wpool = ctx.enter_context(tc.tile_pool(name="wstream", bufs=8))
        w0pool = ctx.enter_context(tc.tile_pool(name="w0c", bufs=4))
        opool = ctx.enter_context(tc.tile_pool(name="oevict", bufs=3))
        psum = ctx.enter_context(tc.tile_pool(name="ps", bufs=8, space="PSUM"))

        # ---- Residents -------------------------------------------------
        # x-shard as 16 ki-pair tiles (1 MB DMAs, 4 KiB/descriptor), sync q.
        xres = []
        for kp in range(KC // 2):
            xk = xpool.tile([P, 2 * S_SH], f32r, tag="xres", name=f"x{kp}")
            nc.sync.dma_start(
                xk[:].rearrange("p (j n) -> p j n", j=2), xt_r[kp])
            xres.append(xk)

        def xs(ki, t):  # rhs slice for contraction chunk ki, token chunk t
            kp, j = divmod(ki, 2)
            off = j * S_SH + t * 512
            return xres[kp][:, off:off + 512]

        # Packed A (host layout [p, ki, lr]): one DMA, 16 KiB/descriptor.
        at_sb = cpool.tile([P, KC * LR], f32r, tag="at")
        nc.scalar.dma_start(at_sb[:], at[:])
        m1_sb = cpool.tile([P, S_SH], f32, tag="m1")
        nc.sync.dma_start(m1_sb[:], m1[:])
        bias_sb = cpool.tile([P, D_OUT // P], f32, tag="bias")
        nc.sync.dma_start(bias_sb[:], bias.rearrange("(j p) -> p j", p=P))
        btf_sb = cpool.tile([P, D_OUT], f32r, tag="btf")
        nc.sync.dma_start(btf_sb[:], btf[:])

        def finalize(po_mc_t, mb, mc):
            """LoRA B-side accumulate + bias eviction + output DMA."""
            j = mb * MC + mc
            osb = opool.tile([P, S_SH], f32, tag="oevict", name=f"o{mb}_{mc}")
            for t in range(TC):
                nc.tensor.matmul(po_mc_t[t][:], btf_sb[:, j * P:(j + 1) * P],
                                 abar[t][:], start=False, stop=True)
                nc.scalar.activation(osb[:, t * 512:(t + 1) * 512],
                                     po_mc_t[t][:],
                                     mybir.ActivationFunctionType.Identity,
                                     bias=bias_sb[:, j:j + 1])
            nc.gpsimd.dma_start(ot_r[j][:], osb[:])

        # ---- Startup ki loop: LoRA-A (2 banks) + mb0 mc0-2 (6 banks) ---
        pa = [psum.tile([P, 512], f32, tag="ps", name=f"pa{t}")
              for t in range(TC)]
        po0 = [[psum.tile([P, 512], f32, tag="ps", name=f"po0_{mc}_{t}")
                for t in range(TC)] for mc in range(3)]
        for ki in range(KC):
            wtile = wpool.tile([P, 512], f32r, tag="wstream", name=f"w0_{ki}")
            nc.scalar.dma_start(wtile[:], wt_r[ki][:, 0:512])
            for t in range(TC):
                nc.tensor.matmul(pa[t][:], at_sb[:, ki * P:(ki + 1) * P],
                                 xs(ki, t), start=(ki == 0), stop=(ki == KC - 1))
            for mc in range(3):
                for t in range(TC):
                    nc.tensor.matmul(po0[mc][t][:],
                                     wtile[:, mc * P:(mc + 1) * P],
                                     xs(ki, t), start=(ki == 0), stop=False)
        abar = []
        for t in range(TC):
            ab = apool.tile([P, 512], f32r, tag="abar", name=f"ab{t}")
            nc.vector.tensor_mul(ab[:], pa[t][:],
                                 m1_sb[:, t * 512:(t + 1) * 512])
            abar.append(ab)
        for mc in range(3):
            finalize(po0[mc], 0, mc)

        # ---- mb0c: deferred mc=3 of block 0 (re-fetch 2 MB of W) -------
        po3 = [psum.tile([P, 512], f32, tag="ps", name=f"po3_{t}")
               for t in range(TC)]
        for ki in range(KC):
            w3 = w0pool.tile([P, P], f32r, tag="w0c", name=f"w3_{ki}")
            nc.scalar.dma_start(w3[:], wt_r[ki][:, 384:512])
            for t in range(TC):
                nc.tensor.matmul(po3[t][:], w3[:], xs(ki, t),
                                 start=(ki == 0), stop=False)
        finalize(po3, 0, 3)

        # ---- Main blocks mb 1..7 ---------------------------------------
        for mb in range(1, MB):
            po = [[psum.tile([P, 512], f32, tag="ps", name=f"po{mb}_{mc}_{t}")
                   for t in range(TC)] for mc in range(MC)]
            for ki in range(KC):
                wtile = wpool.tile([P, 512], f32r, tag="wstream",
                                   name=f"w{mb}_{ki}")
                eng = nc.sync if ki % 2 == 0 else nc.scalar
                eng.dma_start(wtile[:], wt_r[ki][:, mb * 512:(mb + 1) * 512])
                for mc in range(MC):
                    for t in range(TC):
                        nc.tensor.matmul(po[mc][t][:],
                                         wtile[:, mc * P:(mc + 1) * P],
                                         xs(ki, t), start=(ki == 0), stop=False)
            for mc in range(MC):
                finalize(po[mc], mb, mc)

    nc.compile()
    return nc


def get_nc():
    if "nc" not in _CACHE:
        _CACHE["nc"] = _build_nc()
    return _CACHE["nc"]


def prep_in_maps(x, weight, bias, A_buffer, B_buffer, weight_indices):
    """Host-side sharding + layout transforms + fp32r pre-rounding."""
    x = np.asarray(x, dtype=np.float32)
    weight = np.asarray(weight, dtype=np.float32)
    bias = np.asarray(bias, dtype=np.float32)
    A_buffer = np.asarray(A_buffer, dtype=np.float32)
    B_buffer = np.asarray(B_buffer, dtype=np.float32)
    weight_indices = np.asarray(weight_indices)

    wt = round_f32r(weight.T)                              # [D_IN, D_OUT]
    # at[p, ki*LR + lr] = A_stack[lr, ki*P + p]  (PE-ready packed layout)
    a_stack_t = round_f32r(A_buffer.reshape(LR, D_IN).T)   # [D_IN, LR]
    at = np.ascontiguousarray(
        a_stack_t.reshape(KC, P, LR).transpose(1, 0, 2).reshape(P, KC * LR))
    # btf[l*R+r, m] = B_buffer[0, l, m, r] * SCALING
    btf = round_f32r(B_buffer[0].transpose(0, 2, 1).reshape(LR, D_OUT)
                     * SCALING)
    m1_full = np.repeat(
        (np.arange(L)[:, None] == weight_indices[None, :]), R, axis=0
    ).astype(np.float32)                                   # [LR, S]
    xt_full = round_f32r(x).T                              # view [D_IN, S]

    in_maps = []
    for c in range(N_CORES):
        sl = slice(c * S_SH, (c + 1) * S_SH)
        in_maps.append({
            "xt": np.ascontiguousarray(xt_full[:, sl]),
            "wt": wt,
            "at": at,
            "btf": btf,
            "m1": np.ascontiguousarray(m1_full[:, sl]),
            "bias": bias,
        })
    return in_maps


def gather(results):
    out = np.empty((S, D_OUT), dtype=np.float32)
    for c in range(N_CORES):
        out[c * S_SH:(c + 1) * S_SH, :] = results[c]["ot"].T
    return out


def kernel(x, weight, bias, A_buffer, B_buffer, weight_indices):
    nc = get_nc()
    in_maps = prep_in_maps(x, weight, bias, A_buffer, B_buffer, weight_indices)
    res = run_bass_kernel_spmd(nc, in_maps, list(range(N_CORES)))
    return gather(res.results)


# revision 9
# speedup vs baseline: 1.1150x; 1.0158x over previous
"""ColumnParallelLinearWithLoRA kernel for 8 Trainium2 NeuronCores.

Computes out = x @ W.T + bias + 2.0 * lora, where lora routes each token
through one of 8 LoRA adapters (rank 16): lora[s] = B[idx_s] @ (A[idx_s] @ x[s]).

Sharding: data-parallel over tokens (1024 tokens per core). Each core keeps
its x-shard transposed and resident in SBUF, streams the full transposed
weight from HBM exactly once, and computes the LoRA path only for its own
tokens. W/bias/A/B are replicated; the host gathers and transposes the 8
per-core [d_out, tokens] shards back into [S, d_out].

Matmuls run in float32r (fp32 with 11-bit mantissa, 4x the fp32 rate on the
PE): operands are pre-rounded on the host; the LoRA A-projections are rounded
by the DVE when the adapter one-hot mask (built on host) is applied.

The startup x-load (16.8 MB) is overlapped with compute: the LoRA-A
projections (2 PSUM banks) and 3/4 of the first weight block (6 banks) run
in a ki-loop paced by x arrival; the deferred quarter (mc=3) runs as a small
second pass (mb0c) re-fetching its 2 MB of W. DMA queues are split: x/m1/
bias/btf + even-ki W on the Sync HWDGE queue, packed-A + mb0/mb0c/odd-ki W
on the Scalar HWDGE queue, output evictions on GpSimd SWDGE.
"""
from contextlib import ExitStack

import numpy as np

import concourse.bacc as bacc
import concourse.tile as tile
from concourse import mybir
from concourse.bass_utils import run_bass_kernel_spmd

S, D_IN, D_OUT, R, L = 8192, 4096, 4096, 16, 8
SCALING = 2.0
N_CORES = 8
P = 128
S_SH = S // N_CORES          # 1024 tokens per core
LR = L * R                   # 128 stacked adapter rows
KC = D_IN // P               # 32 contraction chunks
TC = S_SH // 512             # 2 token chunks of 512
MB = D_OUT // 512            # 8 output blocks of 512
MC = 4                       # 128-row output chunks per block

f32 = mybir.dt.float32
f32r = mybir.dt.float32r

_CACHE = {}


def round_f32r(a: np.ndarray) -> np.ndarray:
    """Round-to-nearest to fp32r (11 explicit mantissa bits, low 12 bits 0)."""
    u = np.ascontiguousarray(a, dtype=np.float32).view(np.uint32)
    rbit = (u >> np.uint32(12)) & np.uint32(1)
    ur = (u + np.uint32(0x7FF) + rbit) & np.uint32(0xFFFFF000)
    return ur.view(np.float32)


def _build_nc():
    nc = bacc.Bacc("TRN2", target_bir_lowering=False, debug=False,
                   num_devices=N_CORES)
    xt = nc.dram_tensor("xt", [D_IN, S_SH], f32r, kind="ExternalInput").ap()
    wt = nc.dram_tensor("wt", [D_IN, D_OUT], f32r, kind="ExternalInput").ap()
    wt3 = nc.dram_tensor("wt3", [P, KC * P], f32r, kind="ExternalInput").ap()
    at = nc.dram_tensor("at", [P, KC * LR], f32r, kind="ExternalInput").ap()
    btf = nc.dram_tensor("btf", [LR, D_OUT], f32r, kind="ExternalInput").ap()
    m1 = nc.dram_tensor("m1", [LR, S_SH], f32, kind="ExternalInput").ap()
    bias = nc.dram_tensor("bias", [D_OUT], f32, kind="ExternalInput").ap()
    ot = nc.dram_tensor("ot", [D_OUT, S_SH], f32, kind="ExternalOutput").ap()

    # x ki-pair view: [kp, p, j, n] with ki = 2*kp + j
    xt_r = xt.rearrange("(kp j p) n -> kp p j n", p=P, j=2)
    wt_r = wt.rearrange("(ki p) n -> ki p n", p=P)
    ot_r = ot.rearrange("(j p) n -> j p n", p=P)

    with tile.TileContext(nc) as tc, ExitStack() as ctx:
        xpool = ctx.enter_context(tc.tile_pool(name="xres", bufs=KC // 2))
        cpool = ctx.enter_context(tc.tile_pool(name="const", bufs=1))
        apool = ctx.enter_context(tc.tile_pool(name="abar", bufs=TC))
        wpool = ctx.enter_context(tc.tile_pool(name="wstream", bufs=8))
        w0pool = ctx.enter_context(tc.tile_pool(name="w0c", bufs=2))
        opool = ctx.enter_context(tc.tile_pool(name="oevict", bufs=3))
        psum = ctx.enter_context(tc.tile_pool(name="ps", bufs=8, space="PSUM"))

        # ---- Residents -------------------------------------------------
        # x-shard as 16 ki-pair tiles (1 MB DMAs, 4 KiB/descriptor), sync q.
        xres = []
        for kp in range(KC // 2):
            xk = xpool.tile([P, 2 * S_SH], f32r, tag="xres", name=f"x{kp}")
            nc.sync.dma_start(
                xk[:].rearrange("p (j n) -> p j n", j=2), xt_r[kp])
            xres.append(xk)

        def xs(ki, t):  # rhs slice for contraction chunk ki, token chunk t
            kp, j = divmod(ki, 2)
            off = j * S_SH + t * 512
            return xres[kp][:, off:off + 512]

        # Packed A (host layout [p, ki, lr]): 4 DMAs so ki=0 arrives early.
        at_sb = cpool.tile([P, KC * LR], f32r, tag="at")
        for g in range(4):
            sl = slice(g * KC * LR // 4, (g + 1) * KC * LR // 4)
            nc.scalar.dma_start(at_sb[:, sl], at[:, sl])
        m1_sb = cpool.tile([P, S_SH], f32, tag="m1")
        nc.sync.dma_start(m1_sb[:], m1[:])
        bias_sb = cpool.tile([P, D_OUT // P], f32, tag="bias")
        nc.sync.dma_start(bias_sb[:], bias.rearrange("(j p) -> p j", p=P))
        btf_sb = cpool.tile([P, D_OUT], f32r, tag="btf")
        nc.sync.dma_start(btf_sb[:], btf[:])

        def finalize(po_mc_t, mb, mc):
            """LoRA B-side accumulate + bias eviction + output DMA."""
            j = mb * MC + mc
            osb = opool.tile([P, S_SH], f32, tag="oevict", name=f"o{mb}_{mc}")
            for t in range(TC):
                nc.tensor.matmul(po_mc_t[t][:], btf_sb[:, j * P:(j + 1) * P],
                                 abar[t][:], start=False, stop=True)
                nc.scalar.activation(osb[:, t * 512:(t + 1) * 512],
                                     po_mc_t[t][:],
                                     mybir.ActivationFunctionType.Identity,
                                     bias=bias_sb[:, j:j + 1])
            nc.gpsimd.dma_start(ot_r[j][:], osb[:])

        # ---- Startup ki loop: LoRA-A (2 banks) + mb0 mc0-2 (6 banks) ---
        pa = [psum.tile([P, 512], f32, tag="ps", name=f"pa{t}")
              for t in range(TC)]
        po0 = [[psum.tile([P, 512], f32, tag="ps", name=f"po0_{mc}_{t}")
                for t in range(TC)] for mc in range(3)]
        for ki in range(KC):
            wtile = wpool.tile([P, 512], f32r, tag="wstream", name=f"w0_{ki}")
            nc.scalar.dma_start(wtile[:], wt_r[ki][:, 0:512])
            for t in range(TC):
                nc.tensor.matmul(pa[t][:], at_sb[:, ki * P:(ki + 1) * P],
                                 xs(ki, t), start=(ki == 0), stop=(ki == KC - 1))
            for mc in range(3):
                for t in range(TC):
                    nc.tensor.matmul(po0[mc][t][:],
                                     wtile[:, mc * P:(mc + 1) * P],
                                     xs(ki, t), start=(ki == 0), stop=False)
        abar = []
        for t in range(TC):
            ab = apool.tile([P, 512], f32r, tag="abar", name=f"ab{t}")
            nc.vector.tensor_mul(ab[:], pa[t][:],
                                 m1_sb[:, t * 512:(t + 1) * 512])
            abar.append(ab)
        for mc in range(3):
            finalize(po0[mc], 0, mc)

        # ---- mb0c: deferred mc=3 of block 0 (host-packed W re-fetch) ---
        po3 = [psum.tile([P, 512], f32, tag="ps", name=f"po3_{t}")
               for t in range(TC)]
        for g in range(4):  # each chunk covers 8 contraction steps
            w3 = w0pool.tile([P, 8 * P], f32r, tag="w0c", name=f"w3_{g}")
            nc.scalar.dma_start(w3[:], wt3[:, g * 8 * P:(g + 1) * 8 * P])
            for kk in range(8):
                ki = g * 8 + kk
                for t in range(TC):
                    nc.tensor.matmul(po3[t][:], w3[:, kk * P:(kk + 1) * P],
                                     xs(ki, t), start=(ki == 0), stop=False)
        finalize(po3, 0, 3)

        # ---- Main blocks mb 1..7 ---------------------------------------
        for mb in range(1, MB):
            po = [[psum.tile([P, 512], f32, tag="ps", name=f"po{mb}_{mc}_{t}")
                   for t in range(TC)] for mc in range(MC)]
            for ki in range(KC):
                wtile = wpool.tile([P, 512], f32r, tag="wstream",
                                   name=f"w{mb}_{ki}")
                eng = nc.sync if ki % 2 == 0 else nc.scalar
                eng.dma_start(wtile[:], wt_r[ki][:, mb * 512:(mb + 1) * 512])
                for mc in range(MC):
                    for t in range(TC):
                        nc.tensor.matmul(po[mc][t][:],
                                         wtile[:, mc * P:(mc + 1) * P],
                                         xs(ki, t), start=(ki == 0), stop=False)
            for mc in range(MC):
                finalize(po[mc], mb, mc)

    nc.compile()
    return nc


def get_nc():
    if "nc" not in _CACHE:
        _CACHE["nc"] = _build_nc()
    return _CACHE["nc"]


def prep_in_maps(x, weight, bias, A_buffer, B_buffer, weight_indices):
    """Host-side sharding + layout transforms + fp32r pre-rounding."""
    x = np.asarray(x, dtype=np.float32)
    weight = np.asarray(weight, dtype=np.float32)
    bias = np.asarray(bias, dtype=np.float32)
    A_buffer = np.asarray(A_buffer, dtype=np.float32)
    B_buffer = np.asarray(B_buffer, dtype=np.float32)
    weight_indices = np.asarray(weight_indices)

    wt = round_f32r(weight.T)                              # [D_IN, D_OUT]
    # wt3[p, ki*P + n] = wt[ki*P + p, 384 + n]: packed mc=3 cols of block 0
    wt3 = np.ascontiguousarray(
        wt.reshape(KC, P, D_OUT)[:, :, 384:512]
        .transpose(1, 0, 2).reshape(P, KC * P))
    # at[p, ki*LR + lr] = A_stack[lr, ki*P + p]  (PE-ready packed layout)
    a_stack_t = round_f32r(A_buffer.reshape(LR, D_IN).T)   # [D_IN, LR]
    at = np.ascontiguousarray(
        a_stack_t.reshape(KC, P, LR).transpose(1, 0, 2).reshape(P, KC * LR))
    # btf[l*R+r, m] = B_buffer[0, l, m, r] * SCALING
    btf = round_f32r(B_buffer[0].transpose(0, 2, 1).reshape(LR, D_OUT)
                     * SCALING)
    m1_full = np.repeat(
        (np.arange(L)[:, None] == weight_indices[None, :]), R, axis=0
    ).astype(np.float32)                                   # [LR, S]
    xt_full = round_f32r(x).T                              # view [D_IN, S]

    in_maps = []
    for c in range(N_CORES):
        sl = slice(c * S_SH, (c + 1) * S_SH)
        in_maps.append({
            "xt": np.ascontiguousarray(xt_full[:, sl]),
            "wt": wt,
            "wt3": wt3,
            "at": at,
            "btf": btf,
            "m1": np.ascontiguousarray(m1_full[:, sl]),
            "bias": bias,
        })
    return in_maps


def gather(results):
    out = np.empty((S, D_OUT), dtype=np.float32)
    for c in range(N_CORES):
        out[c * S_SH:(c + 1) * S_SH, :] = results[c]["ot"].T
    return out


def kernel(x, weight, bias, A_buffer, B_buffer, weight_indices):
    nc = get_nc()
    in_maps = prep_in_maps(x, weight, bias, A_buffer, B_buffer, weight_indices)
    res = run_bass_kernel_spmd(nc, in_maps, list(range(N_CORES)))
    return gather(res.results)
